# revision 15
# baseline (speedup 1.0000x reference)
"""TP-8 Trainium2 Bass kernel for a LLaDA/Llama transformer block (v3).

Design:
 - norm1 runs on the host: the device receives pre-normalized xn in
   fp8e4m3 (pair-packed for DoubleRow) and x/8 residual in fp16.
 - q/k/v and o projections run fp8e4m3 with perf_mode=DoubleRow
   (0.5 cyc/row on the PE); operands pre-scaled by SW/SX/SA out of
   the fp8 subnormal range, compensated in the PSUM evictions.
 - The residual stream is pre-filled into cc_in (x/8) and the o-proj
   eviction DMA accumulates onto it, so the DVE never touches it.
 - One AllReduce per batch (8.4MB — large chunks amortize the ~33us
   per-collective overhead); batch 0's AR overlaps batch 1's
   attention, batch 1's AR overlaps batch 0's MLP.
 - Emission interleaves independent matmul streams at (st)/(dt,ch)
   granularity via generators, so softmax-latency chains never leave
   the in-order PE queue empty: v-b1 fills attn-b0, o-b0 fills
   attn-b1, norm2-b0 fills o-b1, norm2-b1 + xmh1 prefetch fill the
   ff/up-b0 -> wout-b0 boundary.
 - MLP stays fp16 (fp8 there exceeds the 2e-2 error budget).

Sharding (per sharding_hint): tensor-parallel over 8 cores - q/k/v/ff
sharded on the output-feature axis (4 heads / 1536 ff dims per core),
wo/w_out sharded on the contraction axis; o-projection partials
AllReduced on device (fp16), final projection partials summed on host.
"""

from contextlib import ExitStack

import numpy as np
import ml_dtypes

import concourse.mybir as mybir
import concourse.tile as tile
from concourse import bacc
from concourse.bass_utils import run_bass_kernel_spmd

F32 = mybir.dt.float32
F16 = mybir.dt.float16
F8 = mybir.dt.float8e4
AF = mybir.ActivationFunctionType
ALU = mybir.AluOpType
DR = mybir.MatmulPerfMode.DoubleRow

N_CORES = 8
P = 128
B, T, D, FF = 2, 1024, 4096, 12288
M = B * T            # 2048 tokens
H = 128              # head dim
HALF = 64
QC = D // N_CORES    # 512 per-core q/k/v features (4 heads)
NH = QC // H         # 4 heads per core
FC = FF // N_CORES   # 1536 per-core ff features
NKP = D // P         # 32 K-tiles over D (fp16 granularity)
KP8 = NKP // 2       # 16 fp8 DoubleRow K-pair tiles
NFT = FC // P        # 12 M-tiles over per-core FF
NDT = D // P         # 32 D-tiles
NST = T // P         # 8 sequence tiles per batch
NCH = T // 512       # 2 column chunks per batch
EPS = 1e-05

SW = 16.0            # fp8 weight pre-scale
SX = 8.0             # fp8 xn pre-scale
SA = 32.0            # fp8 attnf pre-scale
IQK = 1.0 / (SW * SX)
IO = 1.0 / (SA * SW)


def _interleave(*gens):
    """Round-robin drive generators to completion."""
    live = list(gens)
    while live:
        nxt = []
        for g in live:
            try:
                next(g)
                nxt.append(g)
            except StopIteration:
                pass
        live = nxt


def _build():
    nc = bacc.Bacc("TRN2", target_bir_lowering=False, num_devices=N_CORES)

    xn8 = nc.declare_dram_parameter("xn8", [KP8, P, 2, M], F8, isOutput=False)
    xs16 = nc.declare_dram_parameter("xs16", [D, M], F16, isOutput=False)
    css = nc.declare_dram_parameter("css", [2, P, M], F16, isOutput=False)
    wq8 = nc.declare_dram_parameter("wq8", [NH, P, KP8, 2, P], F8, isOutput=False)
    wk8 = nc.declare_dram_parameter("wk8", [NH, P, KP8, 2, P], F8, isOutput=False)
    wv8 = nc.declare_dram_parameter("wv8", [P, KP8, 2, QC], F8, isOutput=False)
    wo8 = nc.declare_dram_parameter("wo8", [2, P, 2, NDT, P], F8, isOutput=False)
    wf_t = nc.declare_dram_parameter("wf_t", [NFT, P, NKP, P], F16, isOutput=False)
    wu_t = nc.declare_dram_parameter("wu_t", [NFT, P, NKP, P], F16, isOutput=False)
    wout_t = nc.declare_dram_parameter("wout_t", [NDT, P, NFT, P], F16, isOutput=False)
    y = nc.declare_dram_parameter("y", [D, M], F32, isOutput=True)

    with tile.TileContext(nc) as tc:
        _emit(nc, tc, xn8, xs16, css, wq8, wk8, wv8, wo8, wf_t, wu_t, wout_t, y)
    nc.compile()
    return nc


def _emit(nc, tc, xn8, xs16, css, wq8, wk8, wv8, wo8, wf_t, wu_t, wout_t, y):
    with ExitStack() as top:
        dram_pool = top.enter_context(tc.tile_pool(name="dram", bufs=1, space="DRAM"))
        const = top.enter_context(tc.tile_pool(name="const", bufs=1))
        bc_sp = top.enter_context(tc.tile_pool(name="bc", bufs=1))

        cc_in = [dram_pool.tile([D, T], F16, name=f"cc_in_{b}") for b in range(B)]
        cc_out = [
            dram_pool.tile([D, T], F16, addr_space="Shared", name=f"cc_out_{b}")
            for b in range(B)
        ]
        # pre-fill the residual stream: cc_in = x/8 (o partials accumulate)
        for b in range(B):
            for g in range(4):
                rows = slice(g * (D // 4), (g + 1) * (D // 4))
                nc.sync.dma_start(
                    out=cc_in[b][rows, :],
                    in_=xs16[rows, b * T:(b + 1) * T],
                )

        ones_h = const.tile([P, P], F16)
        nc.vector.memset(ones_h[:], 1.0)
        cc_sb = const.tile([P, M], F16)
        ss_sb = const.tile([P, M], F16)
        nc.sync.dma_start(out=cc_sb[:], in_=css[0])
        nc.sync.dma_start(out=ss_sb[:], in_=css[1])
        eps_sb = const.tile([P, 1], F32)
        nc.vector.memset(eps_sb[:], EPS)
        bcast2 = [bc_sp.tile([P, T], F16, name=f"bcast2_{b}") for b in range(B)]

        # --- pools: two-sided LIFO stacks ---
        es_first = ExitStack()
        first_sp = es_first.enter_context(tc.tile_pool(name="first", bufs=1))
        es_vp = ExitStack()
        v_pp = es_vp.enter_context(
            tc.tile_pool(name="v_p", bufs=1, space="PSUM", side="right")
        )
        es_bp = ExitStack()
        bp_sp = es_bp.enter_context(tc.tile_pool(name="bp", bufs=1, side="right"))
        es_qk = ExitStack()
        qk_sp = es_qk.enter_context(tc.tile_pool(name="qk_s", bufs=1))
        qk_pp = es_qk.enter_context(tc.tile_pool(name="qk_p", bufs=1, space="PSUM"))

        xn8k = []
        for kp in range(KP8):
            xk = first_sp.tile([P, 2, M], F8, name=f"xn8_{kp}")
            nc.sync.dma_start(out=xk[:], in_=xn8[kp])
            xn8k.append(xk)
        wv_sb = first_sp.tile([P, KP8, 2, QC], F8, name="wv_sb")
        nc.sync.dma_start(out=wv_sb[:], in_=wv8[:])

        qf = [[], []]
        kf = [[], []]
        v_sb = [[None] * NST, [None] * NST]
        attnf = [None, None]

        def emit_qk_chain(b, which, wsrc, dst, m):
            gbs = slice(b * T, (b + 1) * T)
            wt = qk_sp.tile([P, KP8, 2, P], F8, tag="wqk", bufs=3,
                            name=f"w{which}_{b}_{m}")
            nc.sync.dma_start(out=wt[:], in_=wsrc[m])
            ps = qk_pp.tile([P, T], F32, tag="qk", bufs=2, name=f"ps{which}_{b}_{m}")
            for ch in range(NCH):
                cs = slice(ch * 512, (ch + 1) * 512)
                gcs = slice(b * T + ch * 512, b * T + (ch + 1) * 512)
                for kp in range(KP8):
                    nc.tensor.matmul(
                        ps[:, cs], wt[:, kp], xn8k[kp][:, :, gcs],
                        start=(kp == 0), stop=(kp == KP8 - 1), perf_mode=DR,
                    )
            main = qk_sp.tile([P, T], F16, tag="rmain", bufs=2,
                              name=f"rm_{which}_{b}_{m}")
            nc.vector.scalar_tensor_tensor(
                main[:], ps[:], IQK, cc_sb[:, gbs], ALU.mult, ALU.mult
            )
            rot = qk_sp.tile([P, T], F16, tag="rrot", bufs=2,
                             name=f"rr_{which}_{b}_{m}")
            nc.vector.scalar_tensor_tensor(
                rot[:HALF], ps[HALF:], -IQK, ss_sb[:HALF, gbs], ALU.mult, ALU.mult
            )
            nc.vector.scalar_tensor_tensor(
                rot[HALF:], ps[:HALF], IQK, ss_sb[HALF:, gbs], ALU.mult, ALU.mult
            )
            out = bp_sp.tile([P, T], F16, name=f"{which}f_{b}_{m}")
            nc.vector.tensor_add(out[:], main[:], rot[:])
            dst.append(out)

        def gen_v(b):
            """v projection for a batch; yields after each kp column."""
            for r in range(NST // 2):
                sts = (2 * r, 2 * r + 1)
                psv = {}
                for st in sts:
                    psv[st] = v_pp.tile([P, QC], F32, tag="vps", bufs=2,
                                        name=f"psv_{b}_{st}")
                for kp in range(KP8):
                    for st in sts:
                        t0 = b * T + st * P
                        nc.tensor.matmul(
                            psv[st][:], xn8k[kp][:, :, t0:t0 + P], wv_sb[:, kp],
                            start=(kp == 0), stop=(kp == KP8 - 1), perf_mode=DR,
                        )
                    if kp % 4 == 3:
                        yield
                for st in sts:
                    vt = bp_sp.tile([P, QC], F16, name=f"v_{b}_{st}")
                    nc.scalar.activation(vt[:], psv[st][:], AF.Copy, scale=IQK)
                    v_sb[b][st] = vt
                yield

        # ---------- phase 1: q/k both batches + v b0 (all fp8 DR) ----------
        vg0 = gen_v(0)
        for m in range(NH):
            emit_qk_chain(0, "q", wq8, qf[0], m)
            emit_qk_chain(0, "k", wk8, kf[0], m)
            for _ in range(5):
                next(vg0, None)
        for m in range(NH):
            emit_qk_chain(1, "q", wq8, qf[1], m)
            emit_qk_chain(1, "k", wk8, kf[1], m)
            for _ in range(5):
                next(vg0, None)
        _interleave(vg0)

        es_qk.close()
        es_att_s = ExitStack()
        att_sp = es_att_s.enter_context(
            tc.tile_pool(name="att_s", bufs=1, side="right")
        )
        es_att_p = ExitStack()
        att_pp = es_att_p.enter_context(
            tc.tile_pool(name="att_p", bufs=1, space="PSUM")
        )

        for b in range(B):
            attnf[b] = bp_sp.tile([P, NH, T], F8, name=f"attnf_{b}")

        def gen_attn_head(b, h):
            """attention for one head; yields after each st step."""
            den_ps = att_pp.tile([P, T], F32, tag="den", bufs=1, name=f"den_{b}_{h}")
            at_ps = att_pp.tile([P, T], F32, tag="at", bufs=1, name=f"at_{b}_{h}")

            def emit_pr(st):
                pr = att_sp.tile([P, T], F16, tag="pr", bufs=4,
                                 name=f"pr_{b}_{h}_{st}")
                for ch in range(NCH):
                    cs = slice(ch * 512, (ch + 1) * 512)
                    lg = att_pp.tile([P, 512], F32, tag="lg", bufs=2,
                                     name=f"lg_{b}_{h}_{st}_{ch}")
                    nc.tensor.matmul(
                        lg[:], kf[b][h][:, st * P:(st + 1) * P], qf[b][h][:, cs],
                        start=True, stop=True,
                    )
                    nc.scalar.activation(pr[:, cs], lg[:], AF.Exp)
                return pr

            prs = [None] * NST
            prs[0] = emit_pr(0)
            yield
            for st in range(NST):
                if st + 1 < NST:
                    prs[st + 1] = emit_pr(st + 1)
                pr = prs[st]
                for ch in range(NCH):
                    cs = slice(ch * 512, (ch + 1) * 512)
                    nc.tensor.matmul(
                        den_ps[:, cs], ones_h[:], pr[:, cs],
                        start=(st == 0), stop=(st == NST - 1),
                    )
                for ch in range(NCH):
                    cs = slice(ch * 512, (ch + 1) * 512)
                    nc.tensor.matmul(
                        at_ps[:, cs], v_sb[b][st][:, h * H:(h + 1) * H], pr[:, cs],
                        start=(st == 0), stop=(st == NST - 1),
                    )
                yield
            for ch in range(NCH):
                cs = slice(ch * 512, (ch + 1) * 512)
                rec = att_sp.tile([P, 512], F32, tag="rec", bufs=4,
                                  name=f"rec_{b}_{h}_{ch}")
                nc.vector.reciprocal(rec[:], den_ps[:, cs])
                nc.vector.scalar_tensor_tensor(
                    attnf[b][:, h, cs], at_ps[:, cs], SA, rec[:],
                    ALU.mult, ALU.mult,
                )
            yield

        def gen_attn(b):
            for h in range(NH):
                yield from gen_attn_head(b, h)

        # ---------- phase 2: attention b0 interleaved with v b1 ----------
        _interleave(gen_attn(0), gen_v(1))

        es_first.close()
        es_vp.close()
        es_o = ExitStack()
        o_sp = es_o.enter_context(tc.tile_pool(name="o_s", bufs=1, side="right"))
        o_pp = es_o.enter_context(
            tc.tile_pool(name="o_p", bufs=1, space="PSUM", side="right")
        )

        wo_sb = []
        for hp in range(2):
            wt = o_sp.tile([P, 2, NDT, P], F8, name=f"wo_sb_{hp}")
            nc.sync.dma_start(out=wt[:], in_=wo8[hp])
            wo_sb.append(wt)

        def gen_o(b):
            """o-projection for a batch; yields after each (dt, ch) chunk;
            fires the batch AllReduce at the end."""
            for dt in range(NDT):
                for ch in range(NCH):
                    cs = slice(ch * 512, (ch + 1) * 512)
                    ps = o_pp.tile([P, 512], F32, tag="o", bufs=2,
                                   name=f"pso_{b}_{dt}_{ch}")
                    for hp in range(2):
                        nc.tensor.matmul(
                            ps[:], wo_sb[hp][:, :, dt, :],
                            attnf[b][:, 2 * hp:2 * hp + 2, cs],
                            start=(hp == 0), stop=(hp == 1), perf_mode=DR,
                        )
                    osb = o_sp.tile([P, 512], F16, tag="osb", bufs=3,
                                    name=f"osb_{b}_{dt}_{ch}")
                    nc.scalar.activation(osb[:], ps[:], AF.Copy, scale=IO)
                    nc.gpsimd.dma_start(
                        out=cc_in[b][dt * P:(dt + 1) * P, cs], in_=osb[:],
                        accum_op=ALU.add,
                    )
                    yield
            nc.gpsimd.collective_compute(
                "AllReduce",
                ALU.add,
                replica_groups=[list(range(N_CORES))],
                ins=[cc_in[b][:, :]],
                outs=[cc_out[b][:, :]],
            )

        # ---------- phase 3: attention b1 interleaved with o-proj b0 ----------
        _interleave(gen_attn(1), gen_o(0))
        es_att_p.close()

        # ---------- phase 4: o-proj b1 interleaved with norm2 b0 ----------
        es_n2 = ExitStack()
        n2_sp = es_n2.enter_context(tc.tile_pool(name="n2_s", bufs=1))
        es_n2p = ExitStack()
        n2_pp = es_n2p.enter_context(tc.tile_pool(name="n2_p", bufs=1, space="PSUM"))

        def gen_norm2(b, ms_ps, pool):
            for kp in range(NKP):
                xk = pool.tile([P, T], F16, tag="xn2", bufs=3, name=f"xn2_{b}_{kp}")
                nc.sync.dma_start(
                    out=xk[:], in_=cc_out[b][kp * P:(kp + 1) * P, :]
                )
                sq = pool.tile([P, T], F16, tag="sq", bufs=3, name=f"sq_{b}_{kp}")
                if kp % 2 == 0:
                    nc.scalar.activation(sq[:], xk[:], AF.Square)
                else:
                    nc.vector.tensor_mul(sq[:], xk[:], xk[:])
                for ch in range(NCH):
                    cs = slice(ch * 512, (ch + 1) * 512)
                    nc.tensor.matmul(
                        ms_ps[:, cs], ones_h[:], sq[:, cs],
                        start=(kp == 0), stop=(kp == NKP - 1),
                    )
                yield

        def finish_norm2(b, ms_ps, pool):
            lnt = pool.tile([P, T], F32, tag="lnt", bufs=1, name=f"lnt_{b}")
            nc.scalar.activation(lnt[:], ms_ps[:], AF.Ln, bias=eps_sb[:],
                                 scale=1.0 / D)
            nc.scalar.activation(bcast2[b][:], lnt[:], AF.Exp, scale=-0.5)

        ms0 = n2_pp.tile([P, T], F32, tag="ms0", bufs=1, name="ms_ps_0")
        _interleave(gen_o(1), gen_norm2(0, ms0, n2_sp))
        finish_norm2(0, ms0, n2_sp)
        es_n2p.close()
        es_n2.close()
        es_o.close()
        es_att_s.close()
        es_bp.close()

        # ---------- phase 5: MLP ff/up batch 0 ----------
        es_mlp0 = ExitStack()
        mlp0_sp = es_mlp0.enter_context(tc.tile_pool(name="mlp0_s", bufs=1))
        es_mlp0w = ExitStack()
        mlp0w_sp = es_mlp0w.enter_context(tc.tile_pool(name="mlp0w_s", bufs=1))
        xmh0 = []
        for kp in range(NKP):
            xk = mlp0_sp.tile([P, T], F16, name=f"xmh0_{kp}")
            nc.sync.dma_start(out=xk[:], in_=cc_out[0][kp * P:(kp + 1) * P, :])
            xmh0.append(xk)

        def emit_ffup(b, xmh, w_sp, h_sp, mlp_pp, hsb):
            ffs = [None] * NFT
            for m in range(NFT):
                for which, wsrc in (("f", wf_t), ("u", wu_t)):
                    wt = w_sp.tile([P, NKP, P], F16, tag="wffu", bufs=3,
                                   name=f"w{which}_{b}_{m}")
                    nc.sync.dma_start(out=wt[:], in_=wsrc[m])
                    ps = mlp_pp.tile([P, T], F32, tag=f"ps_{which}", bufs=2,
                                     name=f"ps{which}_{b}_{m}")
                    for ch in range(NCH):
                        cs = slice(ch * 512, (ch + 1) * 512)
                        for kp in range(NKP):
                            nc.tensor.matmul(
                                ps[:, cs], wt[:, kp, :], xmh[kp][:, cs],
                                start=(kp == 0), stop=(kp == NKP - 1),
                            )
                    nt = w_sp.tile([P, T], F16, tag=f"nrm_{which}", bufs=3,
                                   name=f"nt{which}_{b}_{m}")
                    nc.vector.scalar_tensor_tensor(
                        nt[:], ps[:], 1.0, bcast2[b][:], ALU.mult, ALU.mult
                    )
                    if which == "f":
                        ft = w_sp.tile([P, T], F16, tag="ffs", bufs=3,
                                       name=f"ff_{b}_{m}")
                        nc.scalar.activation(ft[:], nt[:], AF.Silu)
                        ffs[m] = ft
                    else:
                        ht = h_sp.tile([P, T], F16, tag=f"h{m}", name=f"h_{b}_{m}")
                        nc.vector.tensor_mul(ht[:], nt[:], ffs[m][:])
                        hsb.append(ht)

        def emit_wout(b, xmh, hsb, wo2_sp, wo2_pp):
            for dt in range(NDT):
                wt = wo2_sp.tile([P, NFT, P], F16, tag="wot", bufs=3,
                                 name=f"wot_{b}_{dt}")
                nc.sync.dma_start(out=wt[:], in_=wout_t[dt])
                for ch in range(NCH):
                    cs = slice(ch * 512, (ch + 1) * 512)
                    ps = wo2_pp.tile([P, 512], F32, tag="o2", bufs=2,
                                     name=f"pso2_{b}_{dt}_{ch}")
                    for mm in range(NFT):
                        nc.tensor.matmul(
                            ps[:], wt[:, mm, :], hsb[mm][:, cs],
                            start=(mm == 0), stop=(mm == NFT - 1),
                        )
                    ysb = wo2_sp.tile([P, 512], F32, tag="ysb", bufs=3,
                                      name=f"ysb_{b}_{dt}_{ch}")
                    nc.vector.scalar_tensor_tensor(
                        ysb[:], xmh[dt][:, cs], 1.0 / N_CORES, ps[:],
                        ALU.mult, ALU.add,
                    )
                    nc.sync.dma_start(
                        out=y[dt * P:(dt + 1) * P,
                              b * T + ch * 512:b * T + (ch + 1) * 512],
                        in_=ysb[:],
                    )

        es_mlp0p = ExitStack()
        mlp0_pp = es_mlp0p.enter_context(
            tc.tile_pool(name="mlp0_p", bufs=1, space="PSUM")
        )
        hsb0 = []
        emit_ffup(0, xmh0, mlp0w_sp, mlp0_sp, mlp0_pp, hsb0)
        es_mlp0p.close()
        es_mlp0w.close()

        # ---------- phase 5.5: norm2 b1 + xmh1 prefetch (right side) ----------
        es_mlp1 = ExitStack()
        mlp1_sp = es_mlp1.enter_context(
            tc.tile_pool(name="mlp1_s", bufs=1, side="right")
        )
        xmh1 = []
        for kp in range(NKP):
            xk = mlp1_sp.tile([P, T], F16, name=f"xmh1_{kp}")
            nc.sync.dma_start(out=xk[:], in_=cc_out[1][kp * P:(kp + 1) * P, :])
            xmh1.append(xk)

        es_n21 = ExitStack()
        n21_sp = es_n21.enter_context(tc.tile_pool(name="n21_s", bufs=1))
        es_n2b = ExitStack()
        n2b_pp = es_n2b.enter_context(tc.tile_pool(name="n2b_p", bufs=1, space="PSUM"))
        ms1 = n2b_pp.tile([P, T], F32, tag="ms1", bufs=1, name="ms_ps_1")
        _interleave(gen_norm2(1, ms1, n21_sp))
        finish_norm2(1, ms1, n21_sp)
        es_n2b.close()
        es_n21.close()

        # ---------- phase 6: wout b0 ----------
        es_wo20 = ExitStack()
        wo20_sp = es_wo20.enter_context(tc.tile_pool(name="wo20_s", bufs=1))
        wo20_pp = es_wo20.enter_context(
            tc.tile_pool(name="wo20_p", bufs=1, space="PSUM")
        )
        emit_wout(0, xmh0, hsb0, wo20_sp, wo20_pp)
        es_wo20.close()
        es_mlp0.close()

        # ---------- phase 7/8: MLP batch 1 ----------
        es_mlp1w = ExitStack()
        mlp1w_sp = es_mlp1w.enter_context(tc.tile_pool(name="mlp1w_s", bufs=1))
        es_mlp1p = ExitStack()
        mlp1_pp = es_mlp1p.enter_context(
            tc.tile_pool(name="mlp1_p", bufs=1, space="PSUM")
        )
        hsb1 = []
        emit_ffup(1, xmh1, mlp1w_sp, mlp1_sp, mlp1_pp, hsb1)
        es_mlp1p.close()
        es_wo21 = ExitStack()
        wo21_pp = es_wo21.enter_context(
            tc.tile_pool(name="wo21_p", bufs=1, space="PSUM")
        )
        emit_wout(1, xmh1, hsb1, mlp1w_sp, wo21_pp)
        es_wo21.close()
        es_mlp1w.close()
        es_mlp1.close()


_NC_CACHE = {}


def _get_nc():
    if "nc" not in _NC_CACHE:
        _NC_CACHE["nc"] = _build()
    return _NC_CACHE["nc"]


def _host_prep(x, sin, cos, attn_norm_w, ff_norm_w, wq, wk, wv, wo, w_ff, w_up, w_out):
    f16 = np.float16
    f8 = ml_dtypes.float8_e4m3
    x2 = np.asarray(x, np.float32).reshape(M, D)
    xT = np.ascontiguousarray(x2.T)  # [D, M]

    # host norm1: per-token rms scale folded into a pre-normalized xn
    rs1 = 1.0 / np.sqrt((x2 * x2).mean(-1) + EPS)  # [M]
    xn = xT * rs1[None, :]
    # fp8 pair-packed [kp, p, e, t]: contraction k = kp*256 + e*128 + p
    xn8 = np.ascontiguousarray(
        (xn * SX).astype(f8).reshape(KP8, 2, P, M).transpose(0, 2, 1, 3)
    )

    sinT = np.asarray(sin, np.float32).reshape(M, HALF).T
    cosT = np.asarray(cos, np.float32).reshape(M, HALF).T
    cc = np.concatenate([cosT, cosT], axis=0)
    ss = np.concatenate([sinT, sinT], axis=0)
    css = np.stack([cc, ss]).astype(f16)

    anw = np.asarray(attn_norm_w, np.float32)[:, None]
    fnw = np.asarray(ff_norm_w, np.float32)[:, None]
    wqn = (anw * np.asarray(wq, np.float32)) * (H ** -0.5)
    wkn = anw * np.asarray(wk, np.float32)
    wvn = anw * np.asarray(wv, np.float32)
    wfn = fnw * np.asarray(w_ff, np.float32)
    wun = fnw * np.asarray(w_up, np.float32)
    wo_f = np.asarray(wo, np.float32)
    w_out_f = np.asarray(w_out, np.float32)
    xs16 = (xT / N_CORES).astype(f16)

    def pack_qk(w):  # [D, QC] -> [NH, P, KP8, 2, P] fp8, scaled
        return np.ascontiguousarray(
            (w * SW).astype(f8).reshape(KP8, 2, P, NH, P).transpose(3, 2, 0, 1, 4)
        )

    def mtile(w):
        # [K, F] -> [F/P, P, K/P, P] with [m, p, kp, j] = w[kp*P+p, m*P+j]
        K, F = w.shape
        return np.ascontiguousarray(
            w.reshape(K // P, P, F // P, P).transpose(2, 1, 0, 3)
        )

    in_maps = []
    for c in range(N_CORES):
        qs = slice(c * QC, (c + 1) * QC)
        fs = slice(c * FC, (c + 1) * FC)
        wv8 = np.ascontiguousarray(
            (wvn[:, qs] * SW).astype(f8).reshape(KP8, 2, P, QC).transpose(2, 0, 1, 3)
        )
        wo8 = np.ascontiguousarray(
            (wo_f[qs, :] * SW).astype(f8).reshape(2, 2, P, NDT, P)
            .transpose(0, 2, 1, 3, 4)
        )
        in_maps.append(
            {
                "xn8": xn8,
                "xs16": xs16,
                "css": css,
                "wq8": pack_qk(wqn[:, qs]),
                "wk8": pack_qk(wkn[:, qs]),
                "wv8": wv8,
                "wo8": wo8,
                "wf_t": mtile(wfn[:, fs]).astype(f16),
                "wu_t": mtile(wun[:, fs]).astype(f16),
                "wout_t": mtile(w_out_f[fs, :]).astype(f16),
            }
        )
    return in_maps


def kernel(**inputs) -> np.ndarray:
    nc = _get_nc()
    in_maps = _host_prep(**inputs)
    res = run_bass_kernel_spmd(
        nc, in_maps, core_ids=list(range(N_CORES)), trace=False
    )
    acc = res.results[0]["y"].astype(np.float64)
    for c in range(1, N_CORES):
        acc += res.results[c]["y"]
    return np.ascontiguousarray(acc.T).astype(np.float32).reshape(B, T, D)


# revision 26
# speedup vs baseline: 1.0784x; 1.0784x over previous
"""TP-8 Trainium2 Bass kernel for a LLaDA/Llama transformer block (v3).

Design:
 - norm1 runs on the host: the device receives pre-normalized xn in
   fp8e4m3 (pair-packed for DoubleRow) and x/8 residual in fp16.
 - q/k/v and o projections run fp8e4m3 with perf_mode=DoubleRow
   (0.5 cyc/row on the PE); operands pre-scaled by SW/SX/SA out of
   the fp8 subnormal range, compensated in the PSUM evictions.
 - The residual stream is pre-filled into cc_in (x/8) and the o-proj
   eviction DMA accumulates onto it, so the DVE never touches it.
 - One AllReduce per batch (8.4MB — large chunks amortize the ~33us
   per-collective overhead); batch 0's AR overlaps batch 1's
   attention, batch 1's AR overlaps batch 0's MLP.
 - Emission interleaves independent matmul streams at (st)/(dt,ch)
   granularity via generators, so softmax-latency chains never leave
   the in-order PE queue empty: v-b1 fills attn-b0, o-b0 fills
   attn-b1, norm2-b0 fills o-b1, norm2-b1 + xmh1 prefetch fill the
   ff/up-b0 -> wout-b0 boundary.
 - MLP stays fp16 (fp8 there exceeds the 2e-2 error budget).

Sharding (per sharding_hint): tensor-parallel over 8 cores - q/k/v/ff
sharded on the output-feature axis (4 heads / 1536 ff dims per core),
wo/w_out sharded on the contraction axis; o-projection partials
AllReduced on device (fp16), final projection partials summed on host.
"""

from contextlib import ExitStack

import numpy as np
import ml_dtypes

import concourse.mybir as mybir
import concourse.tile as tile
from concourse import bacc
from concourse.bass_utils import run_bass_kernel_spmd

F32 = mybir.dt.float32
F16 = mybir.dt.float16
F8 = mybir.dt.float8e4
AF = mybir.ActivationFunctionType
ALU = mybir.AluOpType
DR = mybir.MatmulPerfMode.DoubleRow

N_CORES = 8
P = 128
B, T, D, FF = 2, 1024, 4096, 12288
M = B * T            # 2048 tokens
H = 128              # head dim
HALF = 64
QC = D // N_CORES    # 512 per-core q/k/v features (4 heads)
NH = QC // H         # 4 heads per core
FC = FF // N_CORES   # 1536 per-core ff features
NKP = D // P         # 32 K-tiles over D (fp16 granularity)
KP8 = NKP // 2       # 16 fp8 DoubleRow K-pair tiles
NFT = FC // P        # 12 M-tiles over per-core FF
NDT = D // P         # 32 D-tiles
NST = T // P         # 8 sequence tiles per batch
NCH = T // 512       # 2 column chunks per batch
EPS = 1e-05

SW = 16.0            # fp8 weight pre-scale
SX = 8.0             # fp8 xn pre-scale
SA = 32.0            # fp8 attnf pre-scale
IQK = 1.0 / (SW * SX)
IO = 1.0 / (SA * SW)


def _interleave(*items):
    """Drive generators to completion round-robin; an item may be a
    (generator, weight) tuple to take `weight` steps per round."""
    live = [[it[0], it[1]] if isinstance(it, tuple) else [it, 1] for it in items]
    while live:
        nxt = []
        for p in live:
            g, w = p
            alive = True
            for _ in range(w):
                try:
                    next(g)
                except StopIteration:
                    alive = False
                    break
            if alive:
                nxt.append(p)
        live = nxt


def _build():
    nc = bacc.Bacc("TRN2", target_bir_lowering=False, num_devices=N_CORES)

    xn8 = nc.declare_dram_parameter("xn8", [KP8, P, 2, M], F8, isOutput=False)
    xs16 = nc.declare_dram_parameter("xs16", [D, M], F16, isOutput=False)
    css = nc.declare_dram_parameter("css", [2, P, M], F16, isOutput=False)
    wq8 = nc.declare_dram_parameter("wq8", [NH, P, KP8, 2, P], F8, isOutput=False)
    wk8 = nc.declare_dram_parameter("wk8", [NH, P, KP8, 2, P], F8, isOutput=False)
    wv8 = nc.declare_dram_parameter("wv8", [P, KP8, 2, QC], F8, isOutput=False)
    wo8 = nc.declare_dram_parameter("wo8", [2, P, 2, NDT, P], F8, isOutput=False)
    wf_t = nc.declare_dram_parameter("wf_t", [NFT, P, NKP, P], F16, isOutput=False)
    wu_t = nc.declare_dram_parameter("wu_t", [NFT, P, NKP, P], F16, isOutput=False)
    wout_t = nc.declare_dram_parameter("wout_t", [NDT, P, NFT, P], F16, isOutput=False)
    y = nc.declare_dram_parameter("y", [D, M], F32, isOutput=True)

    with tile.TileContext(nc) as tc:
        _emit(nc, tc, xn8, xs16, css, wq8, wk8, wv8, wo8, wf_t, wu_t, wout_t, y)
    nc.compile()
    return nc


def _emit(nc, tc, xn8, xs16, css, wq8, wk8, wv8, wo8, wf_t, wu_t, wout_t, y):
    with ExitStack() as top:
        dram_pool = top.enter_context(tc.tile_pool(name="dram", bufs=1, space="DRAM"))
        const = top.enter_context(tc.tile_pool(name="const", bufs=1))
        bc_sp = top.enter_context(tc.tile_pool(name="bc", bufs=1))

        cc_in = [dram_pool.tile([D, T], F16, name=f"cc_in_{b}") for b in range(B)]
        cc_out = [
            [
                dram_pool.tile([D // 2, T], F16, addr_space="Shared",
                               name=f"cc_out_{b}_{k}")
                for k in range(2)
            ]
            for b in range(B)
        ]

        def xmid_rows(b, kp):
            return cc_out[b][kp // (NKP // 2)][(kp % (NKP // 2)) * P:
                                               (kp % (NKP // 2) + 1) * P, :]

        ones_h = const.tile([P, P], F16)
        nc.vector.memset(ones_h[:], 1.0)
        cc_sb = const.tile([P, M], F16)
        ss_sb = const.tile([P, M], F16)
        nc.sync.dma_start(out=cc_sb[:], in_=css[0])
        nc.sync.dma_start(out=ss_sb[:], in_=css[1])
        eps_sb = const.tile([P, 1], F32)
        nc.vector.memset(eps_sb[:], EPS)
        bcast2 = [bc_sp.tile([P, T], F16, name=f"bcast2_{b}") for b in range(B)]

        # --- pools: two-sided LIFO stacks ---
        es_first = ExitStack()
        first_sp = es_first.enter_context(tc.tile_pool(name="first", bufs=1))
        es_vp = ExitStack()
        v_pp = es_vp.enter_context(
            tc.tile_pool(name="v_p", bufs=1, space="PSUM", side="right")
        )
        es_bp = ExitStack()
        bp_sp = es_bp.enter_context(tc.tile_pool(name="bp", bufs=1, side="right"))
        es_qk_s = ExitStack()
        qk_sp = es_qk_s.enter_context(tc.tile_pool(name="qk_s", bufs=1))
        es_qk_p = ExitStack()
        qk_pp = es_qk_p.enter_context(tc.tile_pool(name="qk_p", bufs=1, space="PSUM"))

        xn8k = []
        for kp in range(KP8):
            xk = first_sp.tile([P, 2, M], F8, name=f"xn8_{kp}")
            nc.sync.dma_start(out=xk[:], in_=xn8[kp])
            xn8k.append(xk)
        wv_sb = first_sp.tile([P, KP8, 2, QC], F8, name="wv_sb")
        nc.sync.dma_start(out=wv_sb[:], in_=wv8[:])

        qf = [[], []]
        kf = [[], []]
        v_sb = [[None] * NST, [None] * NST]
        attnf = [None, None]

        def emit_qk_chain(b, which, wsrc, dst, m):
            gbs = slice(b * T, (b + 1) * T)
            wt = qk_sp.tile([P, KP8, 2, P], F8, tag="wqk", bufs=3,
                            name=f"w{which}_{b}_{m}")
            nc.sync.dma_start(out=wt[:], in_=wsrc[m])
            ps = qk_pp.tile([P, T], F32, tag="qk", bufs=2, name=f"ps{which}_{b}_{m}")
            for ch in range(NCH):
                cs = slice(ch * 512, (ch + 1) * 512)
                gcs = slice(b * T + ch * 512, b * T + (ch + 1) * 512)
                for kp in range(KP8):
                    nc.tensor.matmul(
                        ps[:, cs], wt[:, kp], xn8k[kp][:, :, gcs],
                        start=(kp == 0), stop=(kp == KP8 - 1), perf_mode=DR,
                    )
            main = qk_sp.tile([P, T], F16, tag="rmain", bufs=2,
                              name=f"rm_{which}_{b}_{m}")
            nc.vector.scalar_tensor_tensor(
                main[:], ps[:], IQK, cc_sb[:, gbs], ALU.mult, ALU.mult
            )
            rot = qk_sp.tile([P, T], F16, tag="rrot", bufs=2,
                             name=f"rr_{which}_{b}_{m}")
            nc.vector.scalar_tensor_tensor(
                rot[:HALF], ps[HALF:], -IQK, ss_sb[:HALF, gbs], ALU.mult, ALU.mult
            )
            nc.vector.scalar_tensor_tensor(
                rot[HALF:], ps[:HALF], IQK, ss_sb[HALF:, gbs], ALU.mult, ALU.mult
            )
            out = bp_sp.tile([P, T], F16, name=f"{which}f_{b}_{m}")
            nc.vector.tensor_add(out[:], main[:], rot[:])
            dst.append(out)

        def gen_v(b):
            """v projection for a batch; yields after each kp column."""
            for r in range(NST // 2):
                sts = (2 * r, 2 * r + 1)
                psv = {}
                for st in sts:
                    psv[st] = v_pp.tile([P, QC], F32, tag="vps", bufs=2,
                                        name=f"psv_{b}_{st}")
                for kp in range(KP8):
                    for st in sts:
                        t0 = b * T + st * P
                        nc.tensor.matmul(
                            psv[st][:], xn8k[kp][:, :, t0:t0 + P], wv_sb[:, kp],
                            start=(kp == 0), stop=(kp == KP8 - 1), perf_mode=DR,
                        )
                    if kp % 4 == 3:
                        yield
                for st in sts:
                    vt = bp_sp.tile([P, QC], F16, name=f"v_{b}_{st}")
                    nc.scalar.activation(vt[:], psv[st][:], AF.Copy, scale=IQK)
                    v_sb[b][st] = vt
                yield

        # ---------- phase 1: q/k b0 + v both batches (all fp8 DR) ----------
        def gen_vs():
            yield from gen_v(0)
            yield from gen_v(1)

        vg = gen_vs()
        for m in range(NH):
            emit_qk_chain(0, "q", wq8, qf[0], m)
            emit_qk_chain(0, "k", wk8, kf[0], m)
            for _ in range(10):
                next(vg, None)
        _interleave(vg)

        es_qk_p.close()
        es_vp.close()
        es_att_s = ExitStack()
        att_sp = es_att_s.enter_context(
            tc.tile_pool(name="att_s", bufs=1, side="right")
        )
        es_att_p = ExitStack()
        att_pp = es_att_p.enter_context(
            tc.tile_pool(name="att_p", bufs=1, space="PSUM")
        )
        es_qk2 = ExitStack()
        qk2_pp = es_qk2.enter_context(
            tc.tile_pool(name="qk2_p", bufs=1, space="PSUM", side="right")
        )

        def gen_qk_chunked(b):
            """q/k chains with 1-bank psum chunks; yields ~2 per chunk."""
            for which, wsrc, dst in (("q", wq8, qf[b]), ("k", wk8, kf[b])):
                for m in range(NH):
                    wt = qk_sp.tile([P, KP8, 2, P], F8, tag="wqk", bufs=3,
                                    name=f"w{which}_{b}_{m}")
                    nc.sync.dma_start(out=wt[:], in_=wsrc[m])
                    out = bp_sp.tile([P, T], F16, name=f"{which}f_{b}_{m}")
                    for ch in range(NCH):
                        cs = slice(ch * 512, (ch + 1) * 512)
                        gcs = slice(b * T + ch * 512, b * T + (ch + 1) * 512)
                        ps = qk2_pp.tile([P, 512], F32, tag="qk2", bufs=2,
                                         name=f"ps{which}_{b}_{m}_{ch}")
                        for kp in range(KP8):
                            nc.tensor.matmul(
                                ps[:], wt[:, kp], xn8k[kp][:, :, gcs],
                                start=(kp == 0), stop=(kp == KP8 - 1),
                                perf_mode=DR,
                            )
                            if kp == KP8 // 2 - 1:
                                yield
                        main = qk_sp.tile([P, 512], F16, tag="rmain2", bufs=2,
                                          name=f"rm_{which}_{b}_{m}_{ch}")
                        nc.vector.scalar_tensor_tensor(
                            main[:], ps[:], IQK, cc_sb[:, gcs],
                            ALU.mult, ALU.mult,
                        )
                        rot = qk_sp.tile([P, 512], F16, tag="rrot2", bufs=2,
                                         name=f"rr_{which}_{b}_{m}_{ch}")
                        nc.vector.scalar_tensor_tensor(
                            rot[:HALF], ps[HALF:], -IQK, ss_sb[:HALF, gcs],
                            ALU.mult, ALU.mult,
                        )
                        nc.vector.scalar_tensor_tensor(
                            rot[HALF:], ps[:HALF], IQK, ss_sb[HALF:, gcs],
                            ALU.mult, ALU.mult,
                        )
                        nc.vector.tensor_add(out[:, cs], main[:], rot[:])
                        yield
                    dst.append(out)

        for b in range(B):
            attnf[b] = bp_sp.tile([P, NH, T], F8, name=f"attnf_{b}")

        def gen_attn_head(b, h):
            """attention for one head; yields after each st step."""
            den_ps = att_pp.tile([P, T], F32, tag="den", bufs=1, name=f"den_{b}_{h}")
            at_ps = att_pp.tile([P, T], F32, tag="at", bufs=1, name=f"at_{b}_{h}")

            def emit_pr(st):
                pr = att_sp.tile([P, T], F16, tag="pr", bufs=4,
                                 name=f"pr_{b}_{h}_{st}")
                for ch in range(NCH):
                    cs = slice(ch * 512, (ch + 1) * 512)
                    lg = att_pp.tile([P, 512], F32, tag="lg", bufs=2,
                                     name=f"lg_{b}_{h}_{st}_{ch}")
                    nc.tensor.matmul(
                        lg[:], kf[b][h][:, st * P:(st + 1) * P], qf[b][h][:, cs],
                        start=True, stop=True,
                    )
                    nc.scalar.activation(pr[:, cs], lg[:], AF.Exp)
                return pr

            prs = [None] * NST
            prs[0] = emit_pr(0)
            yield
            for st in range(NST):
                if st + 1 < NST:
                    prs[st + 1] = emit_pr(st + 1)
                pr = prs[st]
                for ch in range(NCH):
                    cs = slice(ch * 512, (ch + 1) * 512)
                    nc.tensor.matmul(
                        den_ps[:, cs], ones_h[:], pr[:, cs],
                        start=(st == 0), stop=(st == NST - 1),
                    )
                for ch in range(NCH):
                    cs = slice(ch * 512, (ch + 1) * 512)
                    nc.tensor.matmul(
                        at_ps[:, cs], v_sb[b][st][:, h * H:(h + 1) * H], pr[:, cs],
                        start=(st == 0), stop=(st == NST - 1),
                    )
                yield
            for ch in range(NCH):
                cs = slice(ch * 512, (ch + 1) * 512)
                rec = att_sp.tile([P, 512], F32, tag="rec", bufs=4,
                                  name=f"rec_{b}_{h}_{ch}")
                nc.vector.reciprocal_approx_fast(rec[:], den_ps[:, cs])
                nc.vector.scalar_tensor_tensor(
                    attnf[b][:, h, cs], at_ps[:, cs], SA, rec[:],
                    ALU.mult, ALU.mult,
                )
            yield

        def gen_attn(b):
            for h in range(NH):
                yield from gen_attn_head(b, h)

        # ---------- phase 2: attention b0 interleaved with q/k b1 ----------
        _interleave(gen_attn(0), gen_qk_chunked(1))

        es_qk2.close()
        es_qk_s.close()
        es_first.close()
        es_o = ExitStack()
        o_sp = es_o.enter_context(tc.tile_pool(name="o_s", bufs=1, side="right"))
        o_pp = es_o.enter_context(
            tc.tile_pool(name="o_p", bufs=1, space="PSUM", side="right")
        )

        wo_sb = []
        for hp in range(2):
            wt = o_sp.tile([P, 2, NDT, P], F8, name=f"wo_sb_{hp}")
            nc.sync.dma_start(out=wt[:], in_=wo8[hp])
            wo_sb.append(wt)

        def gen_o(b):
            """o-projection for a batch; yields after each (dt, ch) chunk;
            fires the batch AllReduce in two row-halves."""
            gbs = slice(b * T, (b + 1) * T)
            for dt in range(NDT):
                xt = o_sp.tile([P, T], F16, tag="xs3", bufs=3, name=f"xo_{b}_{dt}")
                nc.sync.dma_start(out=xt[:], in_=xs16[dt * P:(dt + 1) * P, gbs])
                for ch in range(NCH):
                    cs = slice(ch * 512, (ch + 1) * 512)
                    ps = o_pp.tile([P, 512], F32, tag="o", bufs=2,
                                   name=f"pso_{b}_{dt}_{ch}")
                    for hp in range(2):
                        nc.tensor.matmul(
                            ps[:], wo_sb[hp][:, :, dt, :],
                            attnf[b][:, 2 * hp:2 * hp + 2, cs],
                            start=(hp == 0), stop=(hp == 1), perf_mode=DR,
                        )
                    osb = o_sp.tile([P, 512], F16, tag="osb", bufs=3,
                                    name=f"osb_{b}_{dt}_{ch}")
                    nc.vector.scalar_tensor_tensor(
                        osb[:], ps[:], IO, xt[:, cs], ALU.mult, ALU.add
                    )
                    nc.sync.dma_start(
                        out=cc_in[b][dt * P:(dt + 1) * P, cs], in_=osb[:]
                    )
                    yield
                if dt == NDT // 2 - 1 or dt == NDT - 1:
                    k = 0 if dt < NDT // 2 else 1
                    rows = slice(k * (D // 2), (k + 1) * (D // 2))
                    nc.gpsimd.collective_compute(
                        "AllReduce",
                        ALU.add,
                        replica_groups=[list(range(N_CORES))],
                        ins=[cc_in[b][rows, :]],
                        outs=[cc_out[b][k][:, :]],
                    )

        # ---------- phase 3: attention b1 interleaved with o-proj b0 ----------
        _interleave(gen_attn(1), (gen_o(0), 3))
        es_att_p.close()

        # ---------- phase 4: o-proj b1 interleaved with norm2 b0 ----------
        es_n2 = ExitStack()
        n2_sp = es_n2.enter_context(tc.tile_pool(name="n2_s", bufs=1))
        es_n2p = ExitStack()
        n2_pp = es_n2p.enter_context(tc.tile_pool(name="n2_p", bufs=1, space="PSUM"))

        def gen_norm2(b, ms_ps, pool, tiles=None):
            for kp in range(NKP):
                if tiles is None:
                    xk = pool.tile([P, T], F16, tag="xn2", bufs=3,
                                   name=f"xn2_{b}_{kp}")
                    nc.sync.dma_start(out=xk[:], in_=xmid_rows(b, kp))
                else:
                    xk = tiles[kp]
                sq = pool.tile([P, T], F16, tag="sq", bufs=3, name=f"sq_{b}_{kp}")
                if kp % 2 == 0:
                    nc.scalar.activation(sq[:], xk[:], AF.Square)
                else:
                    nc.vector.tensor_mul(sq[:], xk[:], xk[:])
                for ch in range(NCH):
                    cs = slice(ch * 512, (ch + 1) * 512)
                    nc.tensor.matmul(
                        ms_ps[:, cs], ones_h[:], sq[:, cs],
                        start=(kp == 0), stop=(kp == NKP - 1),
                    )
                yield

        def finish_norm2(b, ms_ps, pool):
            lnt = pool.tile([P, T], F32, tag="lnt", bufs=1, name=f"lnt_{b}")
            nc.scalar.activation(lnt[:], ms_ps[:], AF.Ln, bias=eps_sb[:],
                                 scale=1.0 / D)
            nc.scalar.activation(bcast2[b][:], lnt[:], AF.Exp, scale=-0.5)

        ms0 = n2_pp.tile([P, T], F32, tag="ms0", bufs=1, name="ms_ps_0")
        _interleave(gen_o(1), gen_norm2(0, ms0, n2_sp))
        finish_norm2(0, ms0, n2_sp)
        es_n2p.close()
        es_n2.close()
        es_o.close()
        es_att_s.close()
        es_bp.close()

        # ---------- phase 5: MLP ff/up batch 0 ----------
        es_mlp0 = ExitStack()
        mlp0_sp = es_mlp0.enter_context(tc.tile_pool(name="mlp0_s", bufs=1))
        es_mlp0w = ExitStack()
        mlp0w_sp = es_mlp0w.enter_context(tc.tile_pool(name="mlp0w_s", bufs=1))
        xmh0 = []
        for kp in range(NKP):
            xk = mlp0_sp.tile([P, T], F16, name=f"xmh0_{kp}")
            nc.sync.dma_start(out=xk[:], in_=xmid_rows(0, kp))
            xmh0.append(xk)

        def emit_ffup(b, xmh, w_sp, h_sp, mlp_pp, hsb):
            ffs = [None] * NFT
            for m in range(NFT):
                for which, wsrc in (("f", wf_t), ("u", wu_t)):
                    wt = w_sp.tile([P, NKP, P], F16, tag="wffu", bufs=3,
                                   name=f"w{which}_{b}_{m}")
                    nc.sync.dma_start(out=wt[:], in_=wsrc[m])
                    ps = mlp_pp.tile([P, T], F32, tag=f"ps_{which}", bufs=2,
                                     name=f"ps{which}_{b}_{m}")
                    for ch in range(NCH):
                        cs = slice(ch * 512, (ch + 1) * 512)
                        for kp in range(NKP):
                            nc.tensor.matmul(
                                ps[:, cs], wt[:, kp, :], xmh[kp][:, cs],
                                start=(kp == 0), stop=(kp == NKP - 1),
                            )
                    nt = w_sp.tile([P, T], F16, tag=f"nrm_{which}", bufs=3,
                                   name=f"nt{which}_{b}_{m}")
                    nc.vector.scalar_tensor_tensor(
                        nt[:], ps[:], 1.0, bcast2[b][:], ALU.mult, ALU.mult
                    )
                    if which == "f":
                        ft = w_sp.tile([P, T], F16, tag="ffs", bufs=3,
                                       name=f"ff_{b}_{m}")
                        nc.scalar.activation(ft[:], nt[:], AF.Silu)
                        ffs[m] = ft
                    else:
                        ht = h_sp.tile([P, T], F16, tag=f"h{m}", name=f"h_{b}_{m}")
                        nc.vector.tensor_mul(ht[:], nt[:], ffs[m][:])
                        hsb.append(ht)

        def emit_wout(b, xmh, hsb, wo2_sp, wo2_pp):
            for dt in range(NDT):
                wt = wo2_sp.tile([P, NFT, P], F16, tag="wot", bufs=3,
                                 name=f"wot_{b}_{dt}")
                nc.sync.dma_start(out=wt[:], in_=wout_t[dt])
                for ch in range(NCH):
                    cs = slice(ch * 512, (ch + 1) * 512)
                    ps = wo2_pp.tile([P, 512], F32, tag="o2", bufs=2,
                                     name=f"pso2_{b}_{dt}_{ch}")
                    for mm in range(NFT):
                        nc.tensor.matmul(
                            ps[:], wt[:, mm, :], hsb[mm][:, cs],
                            start=(mm == 0), stop=(mm == NFT - 1),
                        )
                    ysb = wo2_sp.tile([P, 512], F32, tag="ysb", bufs=3,
                                      name=f"ysb_{b}_{dt}_{ch}")
                    nc.vector.scalar_tensor_tensor(
                        ysb[:], xmh[dt][:, cs], 1.0 / N_CORES, ps[:],
                        ALU.mult, ALU.add,
                    )
                    nc.sync.dma_start(
                        out=y[dt * P:(dt + 1) * P,
                              b * T + ch * 512:b * T + (ch + 1) * 512],
                        in_=ysb[:],
                    )

        es_mlp0p = ExitStack()
        mlp0_pp = es_mlp0p.enter_context(
            tc.tile_pool(name="mlp0_p", bufs=1, space="PSUM")
        )
        hsb0 = []
        emit_ffup(0, xmh0, mlp0w_sp, mlp0_sp, mlp0_pp, hsb0)
        es_mlp0p.close()
        es_mlp0w.close()

        # ---------- phase 5.5: norm2 b1 + xmh1 prefetch (right side) ----------
        es_mlp1 = ExitStack()
        mlp1_sp = es_mlp1.enter_context(
            tc.tile_pool(name="mlp1_s", bufs=1, side="right")
        )
        xmh1 = []
        for kp in range(NKP):
            xk = mlp1_sp.tile([P, T], F16, name=f"xmh1_{kp}")
            nc.sync.dma_start(out=xk[:], in_=xmid_rows(1, kp))
            xmh1.append(xk)

        es_n21 = ExitStack()
        n21_sp = es_n21.enter_context(tc.tile_pool(name="n21_s", bufs=1))
        es_n2b = ExitStack()
        n2b_pp = es_n2b.enter_context(tc.tile_pool(name="n2b_p", bufs=1, space="PSUM"))
        ms1 = n2b_pp.tile([P, T], F32, tag="ms1", bufs=1, name="ms_ps_1")
        _interleave(gen_norm2(1, ms1, n21_sp, tiles=xmh1))
        finish_norm2(1, ms1, n21_sp)
        es_n2b.close()
        es_n21.close()

        # ---------- phase 6: wout b0 ----------
        es_wo20 = ExitStack()
        wo20_sp = es_wo20.enter_context(tc.tile_pool(name="wo20_s", bufs=1))
        wo20_pp = es_wo20.enter_context(
            tc.tile_pool(name="wo20_p", bufs=1, space="PSUM")
        )
        emit_wout(0, xmh0, hsb0, wo20_sp, wo20_pp)
        es_wo20.close()
        es_mlp0.close()

        # ---------- phase 7/8: MLP batch 1 ----------
        es_mlp1w = ExitStack()
        mlp1w_sp = es_mlp1w.enter_context(tc.tile_pool(name="mlp1w_s", bufs=1))
        es_mlp1p = ExitStack()
        mlp1_pp = es_mlp1p.enter_context(
            tc.tile_pool(name="mlp1_p", bufs=1, space="PSUM")
        )
        hsb1 = []
        emit_ffup(1, xmh1, mlp1w_sp, mlp1_sp, mlp1_pp, hsb1)
        es_mlp1p.close()
        es_wo21 = ExitStack()
        wo21_pp = es_wo21.enter_context(
            tc.tile_pool(name="wo21_p", bufs=1, space="PSUM")
        )
        emit_wout(1, xmh1, hsb1, mlp1w_sp, wo21_pp)
        es_wo21.close()
        es_mlp1w.close()
        es_mlp1.close()


_NC_CACHE = {}


def _get_nc():
    if "nc" not in _NC_CACHE:
        _NC_CACHE["nc"] = _build()
    return _NC_CACHE["nc"]


def _host_prep(x, sin, cos, attn_norm_w, ff_norm_w, wq, wk, wv, wo, w_ff, w_up, w_out):
    f16 = np.float16
    f8 = ml_dtypes.float8_e4m3
    x2 = np.asarray(x, np.float32).reshape(M, D)
    xT = np.ascontiguousarray(x2.T)  # [D, M]

    # host norm1: per-token rms scale folded into a pre-normalized xn
    rs1 = 1.0 / np.sqrt((x2 * x2).mean(-1) + EPS)  # [M]
    xn = xT * rs1[None, :]
    # fp8 pair-packed [kp, p, e, t]: contraction k = kp*256 + e*128 + p
    xn8 = np.ascontiguousarray(
        (xn * SX).astype(f8).reshape(KP8, 2, P, M).transpose(0, 2, 1, 3)
    )

    sinT = np.asarray(sin, np.float32).reshape(M, HALF).T
    cosT = np.asarray(cos, np.float32).reshape(M, HALF).T
    cc = np.concatenate([cosT, cosT], axis=0)
    ss = np.concatenate([sinT, sinT], axis=0)
    css = np.stack([cc, ss]).astype(f16)

    anw = np.asarray(attn_norm_w, np.float32)[:, None]
    fnw = np.asarray(ff_norm_w, np.float32)[:, None]
    wqn = (anw * np.asarray(wq, np.float32)) * (H ** -0.5)
    wkn = anw * np.asarray(wk, np.float32)
    wvn = anw * np.asarray(wv, np.float32)
    wfn = fnw * np.asarray(w_ff, np.float32)
    wun = fnw * np.asarray(w_up, np.float32)
    wo_f = np.asarray(wo, np.float32)
    w_out_f = np.asarray(w_out, np.float32)
    xs16 = (xT / N_CORES).astype(f16)

    def pack_qk(w):  # [D, QC] -> [NH, P, KP8, 2, P] fp8, scaled
        return np.ascontiguousarray(
            (w * SW).astype(f8).reshape(KP8, 2, P, NH, P).transpose(3, 2, 0, 1, 4)
        )

    def mtile(w):
        # [K, F] -> [F/P, P, K/P, P] with [m, p, kp, j] = w[kp*P+p, m*P+j]
        K, F = w.shape
        return np.ascontiguousarray(
            w.reshape(K // P, P, F // P, P).transpose(2, 1, 0, 3)
        )

    in_maps = []
    for c in range(N_CORES):
        qs = slice(c * QC, (c + 1) * QC)
        fs = slice(c * FC, (c + 1) * FC)
        wv8 = np.ascontiguousarray(
            (wvn[:, qs] * SW).astype(f8).reshape(KP8, 2, P, QC).transpose(2, 0, 1, 3)
        )
        wo8 = np.ascontiguousarray(
            (wo_f[qs, :] * SW).astype(f8).reshape(2, 2, P, NDT, P)
            .transpose(0, 2, 1, 3, 4)
        )
        in_maps.append(
            {
                "xn8": xn8,
                "xs16": xs16,
                "css": css,
                "wq8": pack_qk(wqn[:, qs]),
                "wk8": pack_qk(wkn[:, qs]),
                "wv8": wv8,
                "wo8": wo8,
                "wf_t": mtile(wfn[:, fs]).astype(f16),
                "wu_t": mtile(wun[:, fs]).astype(f16),
                "wout_t": mtile(w_out_f[fs, :]).astype(f16),
            }
        )
    return in_maps


def kernel(**inputs) -> np.ndarray:
    nc = _get_nc()
    in_maps = _host_prep(**inputs)
    res = run_bass_kernel_spmd(
        nc, in_maps, core_ids=list(range(N_CORES)), trace=False
    )
    acc = res.results[0]["y"].astype(np.float64)
    for c in range(1, N_CORES):
        acc += res.results[c]["y"]
    return np.ascontiguousarray(acc.T).astype(np.float32).reshape(B, T, D)


# revision 27
# speedup vs baseline: 1.0937x; 1.0142x over previous
"""TP-8 Trainium2 Bass kernel for a LLaDA/Llama transformer block (v3).

Design:
 - norm1 runs on the host: the device receives pre-normalized xn in
   fp8e4m3 (pair-packed for DoubleRow) and x/8 residual in fp16.
 - q/k/v and o projections run fp8e4m3 with perf_mode=DoubleRow
   (0.5 cyc/row on the PE); operands pre-scaled by SW/SX/SA out of
   the fp8 subnormal range, compensated in the PSUM evictions.
 - The residual stream is pre-filled into cc_in (x/8) and the o-proj
   eviction DMA accumulates onto it, so the DVE never touches it.
 - One AllReduce per batch (8.4MB — large chunks amortize the ~33us
   per-collective overhead); batch 0's AR overlaps batch 1's
   attention, batch 1's AR overlaps batch 0's MLP.
 - Emission interleaves independent matmul streams at (st)/(dt,ch)
   granularity via generators, so softmax-latency chains never leave
   the in-order PE queue empty: v-b1 fills attn-b0, o-b0 fills
   attn-b1, norm2-b0 fills o-b1, norm2-b1 + xmh1 prefetch fill the
   ff/up-b0 -> wout-b0 boundary.
 - MLP stays fp16 (fp8 there exceeds the 2e-2 error budget).

Sharding (per sharding_hint): tensor-parallel over 8 cores - q/k/v/ff
sharded on the output-feature axis (4 heads / 1536 ff dims per core),
wo/w_out sharded on the contraction axis; o-projection partials
AllReduced on device (fp16), final projection partials summed on host.
"""

from contextlib import ExitStack

import numpy as np
import ml_dtypes

import concourse.mybir as mybir
import concourse.tile as tile
from concourse import bacc
from concourse.bass_utils import run_bass_kernel_spmd

F32 = mybir.dt.float32
F16 = mybir.dt.float16
F8 = mybir.dt.float8e4
AF = mybir.ActivationFunctionType
ALU = mybir.AluOpType
DR = mybir.MatmulPerfMode.DoubleRow

N_CORES = 8
P = 128
B, T, D, FF = 2, 1024, 4096, 12288
M = B * T            # 2048 tokens
H = 128              # head dim
HALF = 64
QC = D // N_CORES    # 512 per-core q/k/v features (4 heads)
NH = QC // H         # 4 heads per core
FC = FF // N_CORES   # 1536 per-core ff features
NKP = D // P         # 32 K-tiles over D (fp16 granularity)
KP8 = NKP // 2       # 16 fp8 DoubleRow K-pair tiles
NFT = FC // P        # 12 M-tiles over per-core FF
NDT = D // P         # 32 D-tiles
NST = T // P         # 8 sequence tiles per batch
NCH = T // 512       # 2 column chunks per batch
EPS = 1e-05

SW = 16.0            # fp8 weight pre-scale
SX = 8.0             # fp8 xn pre-scale
SA = 32.0            # fp8 attnf pre-scale
IQK = 1.0 / (SW * SX)
IO = 1.0 / (SA * SW)


def _interleave(*items):
    """Drive generators to completion round-robin; an item may be a
    (generator, weight) tuple to take `weight` steps per round."""
    live = [[it[0], it[1]] if isinstance(it, tuple) else [it, 1] for it in items]
    while live:
        nxt = []
        for p in live:
            g, w = p
            alive = True
            for _ in range(w):
                try:
                    next(g)
                except StopIteration:
                    alive = False
                    break
            if alive:
                nxt.append(p)
        live = nxt


def _build():
    nc = bacc.Bacc("TRN2", target_bir_lowering=False, num_devices=N_CORES)

    xn8 = nc.declare_dram_parameter("xn8", [KP8, P, 2, M], F8, isOutput=False)
    xs16 = nc.declare_dram_parameter("xs16", [D, M], F16, isOutput=False)
    css = nc.declare_dram_parameter("css", [2, P, M], F16, isOutput=False)
    wq8 = nc.declare_dram_parameter("wq8", [NH, P, KP8, 2, P], F8, isOutput=False)
    wk8 = nc.declare_dram_parameter("wk8", [NH, P, KP8, 2, P], F8, isOutput=False)
    wv8 = nc.declare_dram_parameter("wv8", [P, KP8, 2, QC], F8, isOutput=False)
    wo8 = nc.declare_dram_parameter("wo8", [2, P, 2, NDT, P], F8, isOutput=False)
    wf_t = nc.declare_dram_parameter("wf_t", [NFT, P, NKP, P], F16, isOutput=False)
    wu_t = nc.declare_dram_parameter("wu_t", [NFT, P, NKP, P], F16, isOutput=False)
    wout_t = nc.declare_dram_parameter("wout_t", [NDT, P, NFT, P], F16, isOutput=False)
    y = nc.declare_dram_parameter("y", [D, M], F32, isOutput=True)

    with tile.TileContext(nc) as tc:
        _emit(nc, tc, xn8, xs16, css, wq8, wk8, wv8, wo8, wf_t, wu_t, wout_t, y)
    nc.compile()
    return nc


def _emit(nc, tc, xn8, xs16, css, wq8, wk8, wv8, wo8, wf_t, wu_t, wout_t, y):
    with ExitStack() as top:
        dram_pool = top.enter_context(tc.tile_pool(name="dram", bufs=1, space="DRAM"))
        const = top.enter_context(tc.tile_pool(name="const", bufs=1))
        bc_sp = top.enter_context(tc.tile_pool(name="bc", bufs=1))

        cc_in = [dram_pool.tile([D, T], F16, name=f"cc_in_{b}") for b in range(B)]
        cc_out = [
            [
                dram_pool.tile([D // 2, T], F16, addr_space="Shared",
                               name=f"cc_out_{b}_{k}")
                for k in range(2)
            ]
            for b in range(B)
        ]

        def xmid_rows(b, kp):
            return cc_out[b][kp // (NKP // 2)][(kp % (NKP // 2)) * P:
                                               (kp % (NKP // 2) + 1) * P, :]

        ones_h = const.tile([P, P], F16)
        nc.vector.memset(ones_h[:], 1.0)
        cc_sb = const.tile([P, M], F16)
        ss_sb = const.tile([P, M], F16)
        nc.sync.dma_start(out=cc_sb[:], in_=css[0])
        nc.sync.dma_start(out=ss_sb[:], in_=css[1])
        eps_sb = const.tile([P, 1], F32)
        nc.vector.memset(eps_sb[:], EPS)
        bcast2 = [bc_sp.tile([P, T], F16, name=f"bcast2_{b}") for b in range(B)]

        # --- pools: two-sided LIFO stacks ---
        es_first = ExitStack()
        first_sp = es_first.enter_context(tc.tile_pool(name="first", bufs=1))
        es_vp = ExitStack()
        v_pp = es_vp.enter_context(
            tc.tile_pool(name="v_p", bufs=1, space="PSUM", side="right")
        )
        es_bp = ExitStack()
        bp_sp = es_bp.enter_context(tc.tile_pool(name="bp", bufs=1, side="right"))
        es_qk_s = ExitStack()
        qk_sp = es_qk_s.enter_context(tc.tile_pool(name="qk_s", bufs=1))
        es_qk_p = ExitStack()
        qk_pp = es_qk_p.enter_context(tc.tile_pool(name="qk_p", bufs=1, space="PSUM"))

        xn8k = []
        for kp in range(KP8):
            xk = first_sp.tile([P, 2, M], F8, name=f"xn8_{kp}")
            nc.sync.dma_start(out=xk[:], in_=xn8[kp])
            xn8k.append(xk)
        wv_sb = first_sp.tile([P, KP8, 2, QC], F8, name="wv_sb")
        nc.sync.dma_start(out=wv_sb[:], in_=wv8[:])

        qf = [[], []]
        kf = [[], []]
        v_sb = [[None] * NST, [None] * NST]
        attnf = [None, None]

        def emit_qk_chain(b, which, wsrc, dst, m):
            gbs = slice(b * T, (b + 1) * T)
            wt = qk_sp.tile([P, KP8, 2, P], F8, tag="wqk", bufs=3,
                            name=f"w{which}_{b}_{m}")
            nc.sync.dma_start(out=wt[:], in_=wsrc[m])
            ps = qk_pp.tile([P, T], F32, tag="qk", bufs=2, name=f"ps{which}_{b}_{m}")
            for ch in range(NCH):
                cs = slice(ch * 512, (ch + 1) * 512)
                gcs = slice(b * T + ch * 512, b * T + (ch + 1) * 512)
                for kp in range(KP8):
                    nc.tensor.matmul(
                        ps[:, cs], wt[:, kp], xn8k[kp][:, :, gcs],
                        start=(kp == 0), stop=(kp == KP8 - 1), perf_mode=DR,
                    )
            main = qk_sp.tile([P, T], F16, tag="rmain", bufs=2,
                              name=f"rm_{which}_{b}_{m}")
            nc.vector.scalar_tensor_tensor(
                main[:], ps[:], IQK, cc_sb[:, gbs], ALU.mult, ALU.mult
            )
            rot = qk_sp.tile([P, T], F16, tag="rrot", bufs=2,
                             name=f"rr_{which}_{b}_{m}")
            nc.vector.scalar_tensor_tensor(
                rot[:HALF], ps[HALF:], -IQK, ss_sb[:HALF, gbs], ALU.mult, ALU.mult
            )
            nc.vector.scalar_tensor_tensor(
                rot[HALF:], ps[:HALF], IQK, ss_sb[HALF:, gbs], ALU.mult, ALU.mult
            )
            out = bp_sp.tile([P, T], F16, name=f"{which}f_{b}_{m}")
            nc.vector.tensor_add(out[:], main[:], rot[:])
            dst.append(out)

        def gen_v(b):
            """v projection for a batch; yields after each kp column."""
            for r in range(NST // 2):
                sts = (2 * r, 2 * r + 1)
                psv = {}
                for st in sts:
                    psv[st] = v_pp.tile([P, QC], F32, tag="vps", bufs=2,
                                        name=f"psv_{b}_{st}")
                for kp in range(KP8):
                    for st in sts:
                        t0 = b * T + st * P
                        nc.tensor.matmul(
                            psv[st][:], xn8k[kp][:, :, t0:t0 + P], wv_sb[:, kp],
                            start=(kp == 0), stop=(kp == KP8 - 1), perf_mode=DR,
                        )
                    if kp % 4 == 3:
                        yield
                for st in sts:
                    vt = bp_sp.tile([P, QC], F16, name=f"v_{b}_{st}")
                    nc.scalar.activation(vt[:], psv[st][:], AF.Copy, scale=IQK)
                    v_sb[b][st] = vt
                yield

        # ---------- phase 1: q/k b0 + v both batches (all fp8 DR) ----------
        def gen_vs():
            yield from gen_v(0)
            yield from gen_v(1)

        vg = gen_vs()
        for m in range(NH):
            emit_qk_chain(0, "q", wq8, qf[0], m)
            emit_qk_chain(0, "k", wk8, kf[0], m)
            for _ in range(10):
                next(vg, None)
        _interleave(vg)

        es_qk_p.close()
        es_vp.close()
        es_att_s = ExitStack()
        att_sp = es_att_s.enter_context(
            tc.tile_pool(name="att_s", bufs=1, side="right")
        )
        es_att_p = ExitStack()
        att_pp = es_att_p.enter_context(
            tc.tile_pool(name="att_p", bufs=1, space="PSUM")
        )
        es_qk2 = ExitStack()
        qk2_pp = es_qk2.enter_context(
            tc.tile_pool(name="qk2_p", bufs=1, space="PSUM", side="right")
        )

        def gen_qk_chunked(b):
            """q/k chains with 1-bank psum chunks; yields ~2 per chunk."""
            for which, wsrc, dst in (("q", wq8, qf[b]), ("k", wk8, kf[b])):
                for m in range(NH):
                    wt = qk_sp.tile([P, KP8, 2, P], F8, tag="wqk", bufs=3,
                                    name=f"w{which}_{b}_{m}")
                    nc.sync.dma_start(out=wt[:], in_=wsrc[m])
                    out = bp_sp.tile([P, T], F16, name=f"{which}f_{b}_{m}")
                    for ch in range(NCH):
                        cs = slice(ch * 512, (ch + 1) * 512)
                        gcs = slice(b * T + ch * 512, b * T + (ch + 1) * 512)
                        ps = qk2_pp.tile([P, 512], F32, tag="qk2", bufs=2,
                                         name=f"ps{which}_{b}_{m}_{ch}")
                        for kp in range(KP8):
                            nc.tensor.matmul(
                                ps[:], wt[:, kp], xn8k[kp][:, :, gcs],
                                start=(kp == 0), stop=(kp == KP8 - 1),
                                perf_mode=DR,
                            )
                            if kp == KP8 // 2 - 1:
                                yield
                        main = qk_sp.tile([P, 512], F16, tag="rmain2", bufs=2,
                                          name=f"rm_{which}_{b}_{m}_{ch}")
                        nc.vector.scalar_tensor_tensor(
                            main[:], ps[:], IQK, cc_sb[:, gcs],
                            ALU.mult, ALU.mult,
                        )
                        rot = qk_sp.tile([P, 512], F16, tag="rrot2", bufs=2,
                                         name=f"rr_{which}_{b}_{m}_{ch}")
                        nc.vector.scalar_tensor_tensor(
                            rot[:HALF], ps[HALF:], -IQK, ss_sb[:HALF, gcs],
                            ALU.mult, ALU.mult,
                        )
                        nc.vector.scalar_tensor_tensor(
                            rot[HALF:], ps[:HALF], IQK, ss_sb[HALF:, gcs],
                            ALU.mult, ALU.mult,
                        )
                        nc.vector.tensor_add(out[:, cs], main[:], rot[:])
                        yield
                    dst.append(out)

        for b in range(B):
            attnf[b] = bp_sp.tile([P, NH, T], F8, name=f"attnf_{b}")

        def gen_attn_head(b, h):
            """attention for one head; yields after each st step."""
            den_ps = att_pp.tile([P, T], F32, tag="den", bufs=1, name=f"den_{b}_{h}")
            at_ps = att_pp.tile([P, T], F32, tag="at", bufs=1, name=f"at_{b}_{h}")

            def emit_pr(st):
                pr = att_sp.tile([P, T], F16, tag="pr", bufs=4,
                                 name=f"pr_{b}_{h}_{st}")
                for ch in range(NCH):
                    cs = slice(ch * 512, (ch + 1) * 512)
                    lg = att_pp.tile([P, 512], F32, tag="lg", bufs=2,
                                     name=f"lg_{b}_{h}_{st}_{ch}")
                    nc.tensor.matmul(
                        lg[:], kf[b][h][:, st * P:(st + 1) * P], qf[b][h][:, cs],
                        start=True, stop=True,
                    )
                    nc.scalar.activation(pr[:, cs], lg[:], AF.Exp)
                return pr

            prs = [None] * NST
            prs[0] = emit_pr(0)
            yield
            for st in range(NST):
                if st + 1 < NST:
                    prs[st + 1] = emit_pr(st + 1)
                pr = prs[st]
                for ch in range(NCH):
                    cs = slice(ch * 512, (ch + 1) * 512)
                    nc.tensor.matmul(
                        den_ps[:, cs], ones_h[:], pr[:, cs],
                        start=(st == 0), stop=(st == NST - 1),
                    )
                for ch in range(NCH):
                    cs = slice(ch * 512, (ch + 1) * 512)
                    nc.tensor.matmul(
                        at_ps[:, cs], v_sb[b][st][:, h * H:(h + 1) * H], pr[:, cs],
                        start=(st == 0), stop=(st == NST - 1),
                    )
                yield
            for ch in range(NCH):
                cs = slice(ch * 512, (ch + 1) * 512)
                rec = att_sp.tile([P, 512], F32, tag="rec", bufs=4,
                                  name=f"rec_{b}_{h}_{ch}")
                nc.vector.reciprocal_approx_fast(rec[:], den_ps[:, cs])
                nc.vector.scalar_tensor_tensor(
                    attnf[b][:, h, cs], at_ps[:, cs], SA, rec[:],
                    ALU.mult, ALU.mult,
                )
            yield

        def gen_attn(b):
            for h in range(NH):
                yield from gen_attn_head(b, h)

        # ---------- phase 2: attention b0 interleaved with q/k b1 ----------
        _interleave(gen_attn(0), gen_qk_chunked(1))

        es_qk2.close()
        es_qk_s.close()
        es_first.close()
        es_o = ExitStack()
        o_sp = es_o.enter_context(tc.tile_pool(name="o_s", bufs=1, side="right"))
        o_pp = es_o.enter_context(
            tc.tile_pool(name="o_p", bufs=1, space="PSUM", side="right")
        )

        wo_sb = []
        for hp in range(2):
            wt = o_sp.tile([P, 2, NDT, P], F8, name=f"wo_sb_{hp}")
            nc.sync.dma_start(out=wt[:], in_=wo8[hp])
            wo_sb.append(wt)

        def gen_o(b):
            """o-projection for a batch; yields after each (dt, ch) chunk;
            fires the batch AllReduce in two row-halves."""
            gbs = slice(b * T, (b + 1) * T)
            for dt in range(NDT):
                xt = o_sp.tile([P, T], F16, tag="xs3", bufs=3, name=f"xo_{b}_{dt}")
                nc.sync.dma_start(out=xt[:], in_=xs16[dt * P:(dt + 1) * P, gbs])
                for ch in range(NCH):
                    cs = slice(ch * 512, (ch + 1) * 512)
                    ps = o_pp.tile([P, 512], F32, tag="o", bufs=2,
                                   name=f"pso_{b}_{dt}_{ch}")
                    for hp in range(2):
                        nc.tensor.matmul(
                            ps[:], wo_sb[hp][:, :, dt, :],
                            attnf[b][:, 2 * hp:2 * hp + 2, cs],
                            start=(hp == 0), stop=(hp == 1), perf_mode=DR,
                        )
                    osb = o_sp.tile([P, 512], F16, tag="osb", bufs=3,
                                    name=f"osb_{b}_{dt}_{ch}")
                    nc.vector.scalar_tensor_tensor(
                        osb[:], ps[:], IO, xt[:, cs], ALU.mult, ALU.add
                    )
                    nc.sync.dma_start(
                        out=cc_in[b][dt * P:(dt + 1) * P, cs], in_=osb[:]
                    )
                    yield
                if dt == NDT // 2 - 1 or dt == NDT - 1:
                    k = 0 if dt < NDT // 2 else 1
                    rows = slice(k * (D // 2), (k + 1) * (D // 2))
                    nc.gpsimd.collective_compute(
                        "AllReduce",
                        ALU.add,
                        replica_groups=[list(range(N_CORES))],
                        ins=[cc_in[b][rows, :]],
                        outs=[cc_out[b][k][:, :]],
                    )

        # ---------- phase 3: attention b1 interleaved with o-proj b0 ----------
        _interleave(gen_attn(1), (gen_o(0), 3))
        es_att_p.close()

        # ---------- phase 4: o-proj b1 interleaved with norm2 b0 ----------
        es_n2 = ExitStack()
        n2_sp = es_n2.enter_context(tc.tile_pool(name="n2_s", bufs=1))
        es_n2p = ExitStack()
        n2_pp = es_n2p.enter_context(tc.tile_pool(name="n2_p", bufs=1, space="PSUM"))

        def gen_norm2(b, ms_ps, pool, tiles=None):
            for kp in range(NKP):
                if tiles is None:
                    xk = pool.tile([P, T], F16, tag="xn2", bufs=3,
                                   name=f"xn2_{b}_{kp}")
                    nc.sync.dma_start(out=xk[:], in_=xmid_rows(b, kp))
                else:
                    xk = tiles[kp]
                sq = pool.tile([P, T], F16, tag="sq", bufs=3, name=f"sq_{b}_{kp}")
                if kp % 2 == 0:
                    nc.scalar.activation(sq[:], xk[:], AF.Square)
                else:
                    nc.vector.tensor_mul(sq[:], xk[:], xk[:])
                for ch in range(NCH):
                    cs = slice(ch * 512, (ch + 1) * 512)
                    nc.tensor.matmul(
                        ms_ps[:, cs], ones_h[:], sq[:, cs],
                        start=(kp == 0), stop=(kp == NKP - 1),
                    )
                yield

        def finish_norm2(b, ms_ps, pool):
            lnt = pool.tile([P, T], F32, tag="lnt", bufs=1, name=f"lnt_{b}")
            nc.scalar.activation(lnt[:], ms_ps[:], AF.Ln, bias=eps_sb[:],
                                 scale=1.0 / D)
            nc.scalar.activation(bcast2[b][:], lnt[:], AF.Exp, scale=-0.5)

        # o-b1 is dense and ready; norm2-b0 waits on AR-b0 — keep it strictly
        # after o-b1 so it can't block the in-order PE queue.
        ms0 = n2_pp.tile([P, T], F32, tag="ms0", bufs=1, name="ms_ps_0")
        _interleave(gen_o(1))
        _interleave(gen_norm2(0, ms0, n2_sp))
        finish_norm2(0, ms0, n2_sp)
        es_n2p.close()
        es_n2.close()
        es_o.close()
        es_att_s.close()
        es_bp.close()

        # ---------- phase 5: MLP ff/up batch 0 ----------
        es_mlp0 = ExitStack()
        mlp0_sp = es_mlp0.enter_context(tc.tile_pool(name="mlp0_s", bufs=1))
        es_mlp0w = ExitStack()
        mlp0w_sp = es_mlp0w.enter_context(tc.tile_pool(name="mlp0w_s", bufs=1))
        xmh0 = []
        for kp in range(NKP):
            xk = mlp0_sp.tile([P, T], F16, name=f"xmh0_{kp}")
            nc.sync.dma_start(out=xk[:], in_=xmid_rows(0, kp))
            xmh0.append(xk)

        def emit_ffup(b, xmh, w_sp, h_sp, mlp_pp, hsb):
            ffs = [None] * NFT
            for m in range(NFT):
                for which, wsrc in (("f", wf_t), ("u", wu_t)):
                    wt = w_sp.tile([P, NKP, P], F16, tag="wffu", bufs=3,
                                   name=f"w{which}_{b}_{m}")
                    nc.sync.dma_start(out=wt[:], in_=wsrc[m])
                    ps = mlp_pp.tile([P, T], F32, tag=f"ps_{which}", bufs=2,
                                     name=f"ps{which}_{b}_{m}")
                    for ch in range(NCH):
                        cs = slice(ch * 512, (ch + 1) * 512)
                        for kp in range(NKP):
                            nc.tensor.matmul(
                                ps[:, cs], wt[:, kp, :], xmh[kp][:, cs],
                                start=(kp == 0), stop=(kp == NKP - 1),
                            )
                    nt = w_sp.tile([P, T], F16, tag=f"nrm_{which}", bufs=3,
                                   name=f"nt{which}_{b}_{m}")
                    nc.vector.scalar_tensor_tensor(
                        nt[:], ps[:], 1.0, bcast2[b][:], ALU.mult, ALU.mult
                    )
                    if which == "f":
                        ft = w_sp.tile([P, T], F16, tag="ffs", bufs=3,
                                       name=f"ff_{b}_{m}")
                        nc.scalar.activation(ft[:], nt[:], AF.Silu)
                        ffs[m] = ft
                    else:
                        ht = h_sp.tile([P, T], F16, tag=f"h{m}", name=f"h_{b}_{m}")
                        nc.vector.tensor_mul(ht[:], nt[:], ffs[m][:])
                        hsb.append(ht)

        def emit_wout(b, xmh, hsb, wo2_sp, wo2_pp):
            for dt in range(NDT):
                wt = wo2_sp.tile([P, NFT, P], F16, tag="wot", bufs=3,
                                 name=f"wot_{b}_{dt}")
                nc.sync.dma_start(out=wt[:], in_=wout_t[dt])
                for ch in range(NCH):
                    cs = slice(ch * 512, (ch + 1) * 512)
                    ps = wo2_pp.tile([P, 512], F32, tag="o2", bufs=2,
                                     name=f"pso2_{b}_{dt}_{ch}")
                    for mm in range(NFT):
                        nc.tensor.matmul(
                            ps[:], wt[:, mm, :], hsb[mm][:, cs],
                            start=(mm == 0), stop=(mm == NFT - 1),
                        )
                    ysb = wo2_sp.tile([P, 512], F32, tag="ysb", bufs=3,
                                      name=f"ysb_{b}_{dt}_{ch}")
                    nc.vector.scalar_tensor_tensor(
                        ysb[:], xmh[dt][:, cs], 1.0 / N_CORES, ps[:],
                        ALU.mult, ALU.add,
                    )
                    nc.sync.dma_start(
                        out=y[dt * P:(dt + 1) * P,
                              b * T + ch * 512:b * T + (ch + 1) * 512],
                        in_=ysb[:],
                    )

        es_mlp0p = ExitStack()
        mlp0_pp = es_mlp0p.enter_context(
            tc.tile_pool(name="mlp0_p", bufs=1, space="PSUM")
        )
        hsb0 = []
        emit_ffup(0, xmh0, mlp0w_sp, mlp0_sp, mlp0_pp, hsb0)
        es_mlp0p.close()
        es_mlp0w.close()

        # ---------- phase 5.5: norm2 b1 + xmh1 prefetch (right side) ----------
        es_mlp1 = ExitStack()
        mlp1_sp = es_mlp1.enter_context(
            tc.tile_pool(name="mlp1_s", bufs=1, side="right")
        )
        xmh1 = []
        for kp in range(NKP):
            xk = mlp1_sp.tile([P, T], F16, name=f"xmh1_{kp}")
            nc.sync.dma_start(out=xk[:], in_=xmid_rows(1, kp))
            xmh1.append(xk)

        es_n21 = ExitStack()
        n21_sp = es_n21.enter_context(tc.tile_pool(name="n21_s", bufs=1))
        es_n2b = ExitStack()
        n2b_pp = es_n2b.enter_context(tc.tile_pool(name="n2b_p", bufs=1, space="PSUM"))
        ms1 = n2b_pp.tile([P, T], F32, tag="ms1", bufs=1, name="ms_ps_1")
        _interleave(gen_norm2(1, ms1, n21_sp, tiles=xmh1))
        finish_norm2(1, ms1, n21_sp)
        es_n2b.close()
        es_n21.close()

        # ---------- phase 6: wout b0 ----------
        es_wo20 = ExitStack()
        wo20_sp = es_wo20.enter_context(tc.tile_pool(name="wo20_s", bufs=1))
        wo20_pp = es_wo20.enter_context(
            tc.tile_pool(name="wo20_p", bufs=1, space="PSUM")
        )
        emit_wout(0, xmh0, hsb0, wo20_sp, wo20_pp)
        es_wo20.close()
        es_mlp0.close()

        # ---------- phase 7/8: MLP batch 1 ----------
        es_mlp1w = ExitStack()
        mlp1w_sp = es_mlp1w.enter_context(tc.tile_pool(name="mlp1w_s", bufs=1))
        es_mlp1p = ExitStack()
        mlp1_pp = es_mlp1p.enter_context(
            tc.tile_pool(name="mlp1_p", bufs=1, space="PSUM")
        )
        hsb1 = []
        emit_ffup(1, xmh1, mlp1w_sp, mlp1_sp, mlp1_pp, hsb1)
        es_mlp1p.close()
        es_wo21 = ExitStack()
        wo21_pp = es_wo21.enter_context(
            tc.tile_pool(name="wo21_p", bufs=1, space="PSUM")
        )
        emit_wout(1, xmh1, hsb1, mlp1w_sp, wo21_pp)
        es_wo21.close()
        es_mlp1w.close()
        es_mlp1.close()


_NC_CACHE = {}


def _get_nc():
    if "nc" not in _NC_CACHE:
        _NC_CACHE["nc"] = _build()
    return _NC_CACHE["nc"]


def _host_prep(x, sin, cos, attn_norm_w, ff_norm_w, wq, wk, wv, wo, w_ff, w_up, w_out):
    f16 = np.float16
    f8 = ml_dtypes.float8_e4m3
    x2 = np.asarray(x, np.float32).reshape(M, D)
    xT = np.ascontiguousarray(x2.T)  # [D, M]

    # host norm1: per-token rms scale folded into a pre-normalized xn
    rs1 = 1.0 / np.sqrt((x2 * x2).mean(-1) + EPS)  # [M]
    xn = xT * rs1[None, :]
    # fp8 pair-packed [kp, p, e, t]: contraction k = kp*256 + e*128 + p
    xn8 = np.ascontiguousarray(
        (xn * SX).astype(f8).reshape(KP8, 2, P, M).transpose(0, 2, 1, 3)
    )

    sinT = np.asarray(sin, np.float32).reshape(M, HALF).T
    cosT = np.asarray(cos, np.float32).reshape(M, HALF).T
    cc = np.concatenate([cosT, cosT], axis=0)
    ss = np.concatenate([sinT, sinT], axis=0)
    css = np.stack([cc, ss]).astype(f16)

    anw = np.asarray(attn_norm_w, np.float32)[:, None]
    fnw = np.asarray(ff_norm_w, np.float32)[:, None]
    wqn = (anw * np.asarray(wq, np.float32)) * (H ** -0.5)
    wkn = anw * np.asarray(wk, np.float32)
    wvn = anw * np.asarray(wv, np.float32)
    wfn = fnw * np.asarray(w_ff, np.float32)
    wun = fnw * np.asarray(w_up, np.float32)
    wo_f = np.asarray(wo, np.float32)
    w_out_f = np.asarray(w_out, np.float32)
    xs16 = (xT / N_CORES).astype(f16)

    def pack_qk(w):  # [D, QC] -> [NH, P, KP8, 2, P] fp8, scaled
        return np.ascontiguousarray(
            (w * SW).astype(f8).reshape(KP8, 2, P, NH, P).transpose(3, 2, 0, 1, 4)
        )

    def mtile(w):
        # [K, F] -> [F/P, P, K/P, P] with [m, p, kp, j] = w[kp*P+p, m*P+j]
        K, F = w.shape
        return np.ascontiguousarray(
            w.reshape(K // P, P, F // P, P).transpose(2, 1, 0, 3)
        )

    in_maps = []
    for c in range(N_CORES):
        qs = slice(c * QC, (c + 1) * QC)
        fs = slice(c * FC, (c + 1) * FC)
        wv8 = np.ascontiguousarray(
            (wvn[:, qs] * SW).astype(f8).reshape(KP8, 2, P, QC).transpose(2, 0, 1, 3)
        )
        wo8 = np.ascontiguousarray(
            (wo_f[qs, :] * SW).astype(f8).reshape(2, 2, P, NDT, P)
            .transpose(0, 2, 1, 3, 4)
        )
        in_maps.append(
            {
                "xn8": xn8,
                "xs16": xs16,
                "css": css,
                "wq8": pack_qk(wqn[:, qs]),
                "wk8": pack_qk(wkn[:, qs]),
                "wv8": wv8,
                "wo8": wo8,
                "wf_t": mtile(wfn[:, fs]).astype(f16),
                "wu_t": mtile(wun[:, fs]).astype(f16),
                "wout_t": mtile(w_out_f[fs, :]).astype(f16),
            }
        )
    return in_maps


def kernel(**inputs) -> np.ndarray:
    nc = _get_nc()
    in_maps = _host_prep(**inputs)
    res = run_bass_kernel_spmd(
        nc, in_maps, core_ids=list(range(N_CORES)), trace=False
    )
    acc = res.results[0]["y"].astype(np.float64)
    for c in range(1, N_CORES):
        acc += res.results[c]["y"]
    return np.ascontiguousarray(acc.T).astype(np.float32).reshape(B, T, D)
